# revision 8
# baseline (speedup 1.0000x reference)
"""Distributed attention kernel for 8 TRN2 NeuronCores.

Sharding: tensor-parallel over heads (2 heads/core, Megatron column split of
w_qkv), attention computed per-core for its heads over all batches, then a
per-batch-half AllToAll redistributes the (transposed) attention output so
each core runs the output projection for 1/8 of the tokens against the full
w_proj.

Layout: everything is kept transposed (d on partitions) so that
  - scores come out as S^T (keys on partitions, queries on free axis),
  - softmax needs no max subtraction (logits ~ N(0,1)),
  - the two heads run as row/col-tiled concurrent matmul pairs using the full
    128-wide PE array.
Compute dtype is bf16 with f32 PSUM accumulation.

v1 restructure (from trace analysis of the 478us baseline):
  - one consolidated DMA trigger per x chunk / weight tensor / A2A buffer
    (the serial ~600ns-per-trigger sync queue was stalling consumer matmuls)
  - V computed transposed like Q/K (stream-bound N=512 matmuls) and moved
    into key-partition layout by DMA xbar transposes instead of 512
    LDWEIGHTS-bound N=128 matmuls
  - softmax denominators: reciprocal_approx_fast directly on SBUF per qi
    strip, paced into the attention steps (the iterative reciprocal behind a
    DRAM round-trip was blocking the vector queue at batch boundaries)
  - AllToAll split into batch halves: half 0 redistributes mid-batch, half 1
    early in the next batch; the projection is token-stationary (N=512
    weight streams, token-major f32 output) so the last batch's tail is only
    one half's norm + A2A + projection
"""

import os
import sys

import numpy as np

for _p in ("/opt/trn_rl_repo", os.path.expanduser("~/.axon_site/_ro/trn_rl_repo")):
    if os.path.isdir(_p) and _p not in sys.path:
        sys.path.insert(0, _p)

import ml_dtypes  # noqa: E402

import concourse.bass as bass  # noqa: E402
from concourse import bacc, mybir  # noqa: E402
import concourse.tile as tile  # noqa: E402
from concourse.bass_utils import run_bass_kernel_spmd  # noqa: E402

B, N, DIM, H = 4, 2048, 1024, 16
HD = DIM // H            # 64 head dim
NCORES = 8
HPC = H // NCORES        # 2 heads per core
HC = HPC * HD            # 128 head-cols per core
T = B * N                # 8192 tokens
HTOK = N // 2            # 1024 tokens per batch half
CTOK = HTOK // NCORES    # 128 tokens per core per half
SCALE = HD ** -0.5

BF16 = mybir.dt.bfloat16
F32 = mybir.dt.float32
EXP = mybir.ActivationFunctionType.Exp

LAST_RESULTS = None  # BassKernelResults of the most recent run (for test.py)


def _build():
    nc = bacc.Bacc(num_devices=NCORES)

    # x^T viewed as [k-block, partition, token]
    x_t = nc.declare_dram_parameter("x_t", [8, 128, T], BF16, isOutput=False)
    w_c = nc.declare_dram_parameter("w_c", [8, 128, 3 * HC], BF16, isOutput=False)
    w_p = nc.declare_dram_parameter("w_p", [8, 128, DIM], BF16, isOutput=False)
    b_p = nc.declare_dram_parameter("b_p", [DIM], F32, isOutput=False)
    # token-major output: [batch, half, my 128 tokens, DIM]
    out_tok = nc.declare_dram_parameter(
        "out_tok", [B, 2, CTOK, DIM], F32, isOutput=True
    )

    with tile.TileContext(nc) as tc:
        with (
            tc.tile_pool(name="persist", bufs=1) as persist,
            tc.tile_pool(name="xin", bufs=3) as xin,
            tc.tile_pool(name="work", bufs=3) as work,
            tc.tile_pool(name="ps_mm", bufs=2, space="PSUM") as ps_mm,
            tc.tile_pool(name="ps_s", bufs=2, space="PSUM") as ps_s,
            tc.tile_pool(name="ps_o", bufs=2, space="PSUM") as ps_o,
            tc.tile_pool(name="dram", bufs=1, space="DRAM") as dram,
        ):
            # ---- persistent SBUF tensors ----
            wqkv_sb = persist.tile([128, 8, 3 * HC], BF16)
            wproj_sb = persist.tile([128, 8, DIM], BF16)
            biasb = persist.tile([128, DIM], F32)     # bias bcast to all rows
            ones_sb = persist.tile([128, 1], BF16)
            # double-buffered by batch parity
            QT = persist.tile([128, 2, N], BF16)
            KT = persist.tile([128, 2, N], BF16)
            Vp = persist.tile([128, 2, 16, HPC, HD], BF16)
            attnT = persist.tile([128, 2, N], BF16)

            # ---- DRAM staging ----
            rden_d = dram.tile([B, 4, 2, 512], BF16)
            ag_in = dram.tile([B, 2, NCORES, HC, CTOK], BF16)
            ag_out = dram.tile([B, 2, NCORES, HC, CTOK], BF16)

            def ap3(base, inner, nblk, blk_stride):
                """[128, nblk, inner] view with per-block stride on the free
                axis of `base` (partition dim copied from base's AP)."""
                return bass.AP(
                    tensor=base.tensor,
                    offset=base.offset,
                    ap=[base.ap[0], [blk_stride, nblk], [1, inner]],
                )

            # ---- initial loads (one trigger each) ----
            nc.sync.dma_start(wqkv_sb, w_c.rearrange("k p c -> p k c"))
            nc.sync.dma_start(
                xin_t0 := xin.tile([128, 8, 1024], BF16, tag="xt", name="xt0"),
                x_t[:, :, 0:1024].rearrange("k p c -> p k c"),
            )
            nc.vector.memset(ones_sb, 1.0)
            # prime the exp table load while DMAs run
            _dummy = work.tile([1, 1], F32, tag="dummy")
            nc.scalar.activation(_dummy, ones_sb[0:1, 0:1], EXP)
            nc.sync.dma_start(
                xin_t1 := xin.tile([128, 8, 1024], BF16, tag="xt", name="xt1"),
                x_t[:, :, 1024:2048].rearrange("k p c -> p k c"),
            )
            nc.sync.dma_start(wproj_sb, w_p.rearrange("k p c -> p k c"))
            bp_ap = b_p[0:DIM]
            nc.sync.dma_start(
                biasb,
                bass.AP(tensor=bp_ap.tensor, offset=bp_ap.offset,
                        ap=[[0, 128], [1, DIM]]),
            )

            xt_tiles = {0: xin_t0, 1: xin_t1}

            # ================= phase builders =================

            def u_xdma(tq):
                def u():
                    xt = xin.tile([128, 8, 1024], BF16, tag="xt", name=f"xt{tq}")
                    nc.sync.dma_start(
                        xt,
                        x_t[:, :, tq * 1024:(tq + 1) * 1024].rearrange(
                            "k p c -> p k c"
                        ),
                    )
                    xt_tiles[tq] = xt
                return u

            def u_qkv(m, tq, nh):
                """Fused unit: full contraction for one 512-token strip of
                Q^T (m=0), K^T (m=1) or V^T (m=2) of chunk tq."""
                bb = tq // 2
                par = bb % 2
                strip = (tq % 2) * 1024 + nh * 512  # within batch

                def u():
                    xt = xt_tiles[tq]
                    pmm = ps_mm.tile([128, 512], F32, tag="mm",
                                     name=f"pq{m}{tq}{nh}")
                    for k in range(8):
                        nc.tensor.matmul(
                            pmm,
                            wqkv_sb[:, k, m * 128:(m + 1) * 128],
                            xt[:, k, nh * 512:(nh + 1) * 512],
                            start=(k == 0),
                            stop=(k == 7),
                        )
                    if m == 0:
                        nc.vector.tensor_copy(
                            QT[:, par, strip:strip + 512], pmm)
                    elif m == 1:
                        nc.vector.tensor_copy(
                            KT[:, par, strip:strip + 512], pmm)
                    else:
                        vt = work.tile([128, 512], BF16, tag="vt", bufs=3,
                                       name=f"vt{tq}{nh}")
                        nc.vector.tensor_copy(vt, pmm)
                        kj0 = strip // 128
                        for j in range(4):
                            nc.sync.dma_start_transpose(
                                Vp[:, par, kj0 + j, :, :],
                                vt[:, j * 128:(j + 1) * 128],
                            )
                return u

            def qkv_units(tq):
                """K,V first (attention consumes all kj tiles in the first
                qi sweep of the next batch), Q strips last."""
                return ([u_qkv(1, tq, nh) for nh in range(2)]
                        + [u_qkv(2, tq, nh) for nh in range(2)]
                        + [u_qkv(0, tq, nh) for nh in range(2)])

            dstage_t = {}  # (b, qi) -> [1,2,512] f32 denominators in SBUF

            def u_recip(b, qi):
                def u():
                    dst = dstage_t.pop((b, qi))
                    rf = work.tile([1, 2, 512], F32, tag="rf", bufs=2)
                    nc.vector.reciprocal_approx_fast(out=rf, in_=dst)
                    rb = work.tile([1, 2, 512], BF16, tag="rb", bufs=2)
                    nc.vector.tensor_copy(rb, rf)
                    nc.sync.dma_start(rden_d[b, qi], rb)
                return u

            def u_bcmul(b, qi):
                par = b % 2

                def u():
                    q0 = qi * 512
                    bc = work.tile([128, 512], BF16, tag="bc", bufs=2)
                    for h in range(HPC):
                        src = rden_d[b, qi, h, :]
                        bcast = bass.AP(tensor=src.tensor, offset=src.offset,
                                        ap=[[0, HD], [1, 512]])
                        nc.sync.dma_start(bc[h * HD:(h + 1) * HD, :], bcast)
                    nc.vector.tensor_mul(
                        attnT[:, par, q0:q0 + 512],
                        attnT[:, par, q0:q0 + 512],
                        bc,
                    )
                return u

            def u_a2a(b, half):
                par = b % 2

                def u():
                    base = attnT[:, par, half * HTOK:(half + 1) * HTOK]
                    nc.sync.dma_start(
                        ag_in[b, half].rearrange("j p c -> p j c"),
                        ap3(base, CTOK, NCORES, CTOK),
                    )
                    nc.gpsimd.collective_compute(
                        "AllToAll", mybir.AluOpType.bypass,
                        replica_groups=[list(range(NCORES))],
                        ins=[ag_in[b, half]], outs=[ag_out[b, half]],
                    )
                return u

            def proj_units(b, half):
                """Token-stationary projection of this core's 128 tokens of
                (b, half): out[tok, od] accumulated over the 8 rank blocks."""
                st = {}

                def u_dma():
                    agT = work.tile([128, 8, CTOK], BF16, tag="agT", bufs=2,
                                    name=f"agT{b}{half}")
                    nc.sync.dma_start(
                        agT, ag_out[b, half].rearrange("j p c -> p j c"))
                    st["agT"] = agT

                def mk_od(oh):
                    def u():
                        agT = st["agT"]
                        pp = ps_mm.tile([128, 512], F32, tag="mm",
                                        name=f"pp{b}{half}{oh}")
                        for r in range(8):
                            nc.tensor.matmul(
                                pp,
                                agT[:, r, :],
                                wproj_sb[:, r, oh * 512:(oh + 1) * 512],
                                start=(r == 0),
                                stop=(r == 7),
                            )
                        ob = work.tile([128, 512], F32, tag="ob", bufs=2,
                                       name=f"ob{b}{half}{oh}")
                        nc.vector.tensor_add(
                            ob, pp, biasb[:, oh * 512:(oh + 1) * 512])
                        nc.sync.dma_start(
                            out_tok[b, half, :, oh * 512:(oh + 1) * 512], ob)
                    return u

                return [u_dma, mk_od(0), mk_od(1)]

            # ================= main loop =================
            for b in range(B):
                par = b % 2
                t0 = 0  # attnT/QT/KT are parity-indexed, not batch-offset

                # -- scheduled inserts: step -> [units] --
                timeline = {}

                def put(step, *us):
                    timeline.setdefault(step, []).extend(us)

                if b == 0:
                    # minimal prologue: K, V (+transposes), Q for the first
                    # 512 tokens so attention step (0, 0) can start
                    u_qkv(1, 0, 0)()
                    u_qkv(2, 0, 0)()
                    u_qkv(0, 0, 0)()
                    paced = ([u_qkv(1, 0, 1), u_qkv(2, 0, 1)]
                             + [u_qkv(1, 1, 0), u_qkv(2, 1, 0)]
                             + [u_qkv(1, 1, 1), u_qkv(2, 1, 1)]
                             + [u_qkv(0, 0, 1), u_qkv(0, 1, 0),
                                u_qkv(0, 1, 1)])
                    put(9, u_xdma(2))
                    put(21, u_xdma(3))
                    paced += qkv_units(2) + qkv_units(3)
                else:
                    paced = []
                    if b + 1 < B:
                        put(0, u_xdma(2 * b + 2))
                        put(2, u_xdma(2 * b + 3))
                        paced += qkv_units(2 * b + 2) + qkv_units(2 * b + 3)
                    # previous batch wind-down: last strip norm, half-1 A2A,
                    # then both halves' projections
                    put(2, u_recip(b - 1, 3))
                    put(4, u_bcmul(b - 1, 3))
                    put(6, u_a2a(b - 1, 1))
                    pu0 = proj_units(b - 1, 0)
                    put(10, pu0[0])
                    put(12, pu0[1])
                    put(14, pu0[2])
                    pu1 = proj_units(b - 1, 1)
                    put(26, pu1[0])
                    put(28, pu1[1])
                    put(30, pu1[2])

                # this batch's own norm + half-0 A2A
                put(17, u_recip(b, 0))
                put(19, u_bcmul(b, 0))
                put(33, u_recip(b, 1))
                put(35, u_bcmul(b, 1))
                put(38, u_a2a(b, 0))
                put(49, u_recip(b, 2))
                put(51, u_bcmul(b, 2))
                if b == B - 1:
                    pu = proj_units(b, 0)
                    put(56, pu[0])
                    put(58, pu[1])
                    put(60, pu[2])

                n_fill = len(paced)
                paced.reverse()  # pop() from the end = original order
                popped = 0

                steps = [(qi, kj) for qi in range(4) for kj in range(16)]
                pS_t = {}
                po_t = {}
                acc_t = {}

                def emit_S(qi, kj):
                    q0 = qi * 512
                    k0 = kj * 128
                    pS = ps_s.tile([128, 2, 512], F32, tag="s",
                                   name=f"pS{b}_{qi}_{kj}")
                    for h in range(HPC):
                        hs = h * HD
                        nc.tensor.matmul(
                            pS[:, h, :],
                            KT[hs:hs + HD, par, k0:k0 + 128],
                            QT[hs:hs + HD, par, q0:q0 + 512],
                            start=True,
                            stop=True,
                        )
                    pS_t[(qi, kj)] = pS

                emit_S(0, 0)
                for it, (qi, kj) in enumerate(steps):
                    q0 = qi * 512
                    if kj == 0:
                        po_t[qi] = ps_o.tile([128, 512], F32, tag="vo",
                                             name=f"po{b}_{qi}")
                        acc_t[qi] = [
                            work.tile([128, 2, 512], BF16, tag=f"acc{a}",
                                      name=f"acc{a}_{b}_{qi}")
                            for a in range(2)
                        ]
                    due = timeline.pop(it, [])
                    if b == 0:
                        target = ((it + 1) if it < 9
                                  else 9 + (it - 8) * (n_fill - 9) // 48)
                    else:
                        target = (it + 1) * n_fill // 56
                    # sandwich filler work around the sem-gated instructions
                    # (S waiting its PSUM slot, V waiting eS) so the in-order
                    # PE queue never idles at a blocked head
                    if due:
                        due[0]()
                        due = due[1:]
                    while paced and popped < min(target, n_fill):
                        paced.pop()()
                        popped += 1
                    if it + 1 < len(steps):
                        emit_S(*steps[it + 1])
                    pS = pS_t.pop((qi, kj))
                    eS = work.tile([128, 2, 512], BF16, tag="es", bufs=4)
                    nc.scalar.activation(eS, pS, EXP, scale=SCALE)
                    for u in due:
                        u()
                    po, acc = po_t[qi], acc_t[qi]
                    for h in range(HPC):
                        nc.tensor.matmul(
                            po[h * HD:(h + 1) * HD, :],
                            Vp[:, par, kj, h, :],
                            eS[:, h, :],
                            start=(kj == 0),
                            stop=(kj == 15),
                        )
                    a = kj // 8
                    if kj % 8 == 0:
                        nc.vector.tensor_copy(acc[a], eS)
                    else:
                        nc.vector.tensor_add(acc[a], acc[a], eS)
                    if kj == 15:
                        # stage numerators (unnormalized, both heads)
                        nc.vector.tensor_copy(
                            attnT[:, par, q0:q0 + 512], po)
                        # denominators: partition-reduce the accumulators
                        nc.vector.tensor_add(acc[0], acc[0], acc[1])
                        dst = work.tile([1, 2, 512], F32, tag="dst", bufs=4,
                                        name=f"dst{b}{qi}")
                        for h in range(HPC):
                            pden = ps_mm.tile([1, 512], F32, tag="mm",
                                              name=f"pden{b}{qi}{h}")
                            nc.tensor.matmul(pden, ones_sb[:, 0:1],
                                             acc[0][:, h, :],
                                             start=True, stop=True)
                            nc.vector.tensor_copy(dst[:, h, :], pden)
                        dstage_t[(b, qi)] = dst
                while paced:
                    paced.pop()()
                for s in sorted(timeline):
                    for u in timeline[s]:
                        u()

            # ---- tail: last batch, second half ----
            u_recip(B - 1, 3)()
            u_bcmul(B - 1, 3)()
            u_a2a(B - 1, 1)()
            for u in proj_units(B - 1, 1):
                u()

    nc.finalize()
    return nc


def kernel(x, w_qkv, w_proj, b_proj):
    global LAST_RESULTS
    bf16 = ml_dtypes.bfloat16

    x_t = np.ascontiguousarray(
        x.reshape(T, DIM).T.astype(bf16).reshape(8, 128, T))
    w_p = np.ascontiguousarray(w_proj.astype(bf16).reshape(8, 128, DIM))
    b_p = np.ascontiguousarray(b_proj.astype(np.float32))

    in_maps = []
    for c in range(NCORES):
        w_c = np.concatenate(
            [
                w_qkv[:, HC * c:HC * (c + 1)],
                w_qkv[:, DIM + HC * c:DIM + HC * (c + 1)],
                w_qkv[:, 2 * DIM + HC * c:2 * DIM + HC * (c + 1)],
            ],
            axis=1,
        ).astype(bf16).reshape(8, 128, 3 * HC)
        in_maps.append(
            {"x_t": x_t, "w_c": np.ascontiguousarray(w_c), "w_p": w_p,
             "b_p": b_p}
        )

    nc = _build()
    LAST_RESULTS = run_bass_kernel_spmd(
        nc, in_maps, core_ids=list(range(NCORES)),
        trace=bool(os.environ.get("KERNEL_TRACE")),
    )

    # core c's out_tok[b, hf] holds tokens [hf*1024 + c*128, +128) of batch b
    out = np.empty((B, N, DIM), dtype=np.float32)
    for c in range(NCORES):
        res = np.asarray(LAST_RESULTS.results[c]["out_tok"], dtype=np.float32)
        for b in range(B):
            for hf in range(2):
                o0 = hf * HTOK + c * CTOK
                out[b, o0:o0 + CTOK, :] = res[b, hf]
    return out


# revision 14
# speedup vs baseline: 1.0457x; 1.0457x over previous
"""Distributed attention kernel for 8 TRN2 NeuronCores.

Sharding: tensor-parallel over heads (2 heads/core, Megatron column split of
w_qkv), attention computed per-core for its heads over all batches, then a
per-batch-half AllToAll redistributes the (transposed) attention output so
each core runs the output projection for 1/8 of the tokens against the full
w_proj.

Layout: everything is kept transposed (d on partitions) so that
  - scores come out as S^T (keys on partitions, queries on free axis),
  - softmax needs no max subtraction (logits ~ N(0,1)),
  - the two heads run as row/col-tiled concurrent matmul pairs using the full
    128-wide PE array.
Compute dtype is bf16 with f32 PSUM accumulation.

v1 restructure (from trace analysis of the 478us baseline):
  - one consolidated DMA trigger per x chunk / weight tensor / A2A buffer
    (the serial ~600ns-per-trigger sync queue was stalling consumer matmuls)
  - V computed transposed like Q/K (stream-bound N=512 matmuls) and moved
    into key-partition layout by DMA xbar transposes instead of 512
    LDWEIGHTS-bound N=128 matmuls
  - softmax denominators: reciprocal_approx_fast directly on SBUF per qi
    strip, paced into the attention steps (the iterative reciprocal behind a
    DRAM round-trip was blocking the vector queue at batch boundaries)
  - AllToAll split into batch halves: half 0 redistributes mid-batch, half 1
    early in the next batch; the projection is token-stationary (N=512
    weight streams, token-major f32 output) so the last batch's tail is only
    one half's norm + A2A + projection
"""

import os
import sys

import numpy as np

for _p in ("/opt/trn_rl_repo", os.path.expanduser("~/.axon_site/_ro/trn_rl_repo")):
    if os.path.isdir(_p) and _p not in sys.path:
        sys.path.insert(0, _p)

import ml_dtypes  # noqa: E402

import concourse.bass as bass  # noqa: E402
from concourse import bacc, mybir  # noqa: E402
import concourse.tile as tile  # noqa: E402
from concourse.bass_utils import run_bass_kernel_spmd  # noqa: E402

B, N, DIM, H = 4, 2048, 1024, 16
HD = DIM // H            # 64 head dim
NCORES = 8
HPC = H // NCORES        # 2 heads per core
HC = HPC * HD            # 128 head-cols per core
T = B * N                # 8192 tokens
HTOK = N // 2            # 1024 tokens per batch half
CTOK = HTOK // NCORES    # 128 tokens per core per half
SCALE = HD ** -0.5

BF16 = mybir.dt.bfloat16
F32 = mybir.dt.float32
EXP = mybir.ActivationFunctionType.Exp

LAST_RESULTS = None  # BassKernelResults of the most recent run (for test.py)


def _build():
    nc = bacc.Bacc(num_devices=NCORES)

    # x^T viewed as [k-block, partition, token]
    x_t = nc.declare_dram_parameter("x_t", [8, 128, T], BF16, isOutput=False)
    w_c = nc.declare_dram_parameter("w_c", [8, 128, 3 * HC], BF16, isOutput=False)
    w_p = nc.declare_dram_parameter("w_p", [8, 128, DIM], BF16, isOutput=False)
    b_p = nc.declare_dram_parameter("b_p", [DIM], F32, isOutput=False)
    # token-major output: [batch, half, my 128 tokens, DIM]
    out_tok = nc.declare_dram_parameter(
        "out_tok", [B, 2, CTOK, DIM], F32, isOutput=True
    )

    with tile.TileContext(nc) as tc:
        with (
            tc.tile_pool(name="persist", bufs=1) as persist,
            tc.tile_pool(name="xin", bufs=3) as xin,
            tc.tile_pool(name="work", bufs=3) as work,
            tc.tile_pool(name="ps_mm", bufs=2, space="PSUM") as ps_mm,
            tc.tile_pool(name="ps_s", bufs=2, space="PSUM") as ps_s,
            tc.tile_pool(name="ps_o", bufs=2, space="PSUM") as ps_o,
            tc.tile_pool(name="dram", bufs=1, space="DRAM") as dram,
        ):
            # ---- persistent SBUF tensors ----
            wqkv_sb = persist.tile([128, 8, 3 * HC], BF16)
            wproj_sb = persist.tile([128, 8, DIM], BF16)
            biasb = persist.tile([128, DIM], F32)     # bias bcast to all rows
            ones_sb = persist.tile([128, 1], BF16)
            # double-buffered by batch parity
            QT = persist.tile([128, 2, N], BF16)
            KT = persist.tile([128, 2, N], BF16)
            Vp = persist.tile([128, 2, 16, HPC, HD], BF16)
            attnT = persist.tile([128, 2, N], BF16)

            # ---- DRAM staging ----
            rden_d = dram.tile([B, 4, 2, 512], BF16)
            ag_in = dram.tile([B, 2, NCORES, HC, CTOK], BF16)
            ag_out = dram.tile([B, 2, NCORES, HC, CTOK], BF16)
            warm_in = dram.tile([NCORES, 16], BF16)
            warm_out = dram.tile([NCORES, 16], BF16)

            def ap3(base, inner, nblk, blk_stride):
                """[128, nblk, inner] view with per-block stride on the free
                axis of `base` (partition dim copied from base's AP)."""
                return bass.AP(
                    tensor=base.tensor,
                    offset=base.offset,
                    ap=[base.ap[0], [blk_stride, nblk], [1, inner]],
                )

            # ---- initial loads (one trigger each) ----
            nc.sync.dma_start(wqkv_sb, w_c.rearrange("k p c -> p k c"))
            nc.sync.dma_start(
                xin_t0 := xin.tile([128, 8, 1024], BF16, tag="xt", name="xt0"),
                x_t[:, :, 0:1024].rearrange("k p c -> p k c"),
            )
            nc.vector.memset(ones_sb, 1.0)
            # prime the exp table load while DMAs run
            _dummy = work.tile([1, 1], F32, tag="dummy")
            nc.scalar.activation(_dummy, ones_sb[0:1, 0:1], EXP)
            nc.sync.dma_start(
                xin_t1 := xin.tile([128, 8, 1024], BF16, tag="xt", name="xt1"),
                x_t[:, :, 1024:2048].rearrange("k p c -> p k c"),
            )
            nc.sync.dma_start(wproj_sb, w_p.rearrange("k p c -> p k c"))
            bp_ap = b_p[0:DIM]
            nc.sync.dma_start(
                biasb,
                bass.AP(tensor=bp_ap.tensor, offset=bp_ap.offset,
                        ap=[[0, 128], [1, DIM]]),
            )
            # warm up the collective path (ENCD staging + first-call cost)
            # concurrently with the batch-0 prologue
            nc.gpsimd.collective_compute(
                "AllToAll", mybir.AluOpType.bypass,
                replica_groups=[list(range(NCORES))],
                ins=[warm_in], outs=[warm_out],
            )

            xt_tiles = {0: xin_t0, 1: xin_t1}

            # ================= phase builders =================

            def u_xdma(tq):
                def u():
                    xt = xin.tile([128, 8, 1024], BF16, tag="xt", name=f"xt{tq}")
                    nc.sync.dma_start(
                        xt,
                        x_t[:, :, tq * 1024:(tq + 1) * 1024].rearrange(
                            "k p c -> p k c"
                        ),
                    )
                    xt_tiles[tq] = xt
                return u

            def u_qkv(m, tq, nh):
                """Fused unit: full contraction for one 512-token strip of
                Q^T (m=0), K^T (m=1) or V^T (m=2) of chunk tq."""
                bb = tq // 2
                par = bb % 2
                strip = (tq % 2) * 1024 + nh * 512  # within batch

                def u():
                    xt = xt_tiles[tq]
                    pmm = ps_mm.tile([128, 512], F32, tag="mm",
                                     name=f"pq{m}{tq}{nh}")
                    for k in range(8):
                        nc.tensor.matmul(
                            pmm,
                            wqkv_sb[:, k, m * 128:(m + 1) * 128],
                            xt[:, k, nh * 512:(nh + 1) * 512],
                            start=(k == 0),
                            stop=(k == 7),
                        )
                    if m == 0:
                        nc.vector.tensor_copy(
                            QT[:, par, strip:strip + 512], pmm)
                    elif m == 1:
                        nc.vector.tensor_copy(
                            KT[:, par, strip:strip + 512], pmm)
                    else:
                        vt = work.tile([128, 512], BF16, tag="vt", bufs=3,
                                       name=f"vt{tq}{nh}")
                        nc.vector.tensor_copy(vt, pmm)
                        vt_tiles[(tq, nh)] = vt
                return u

            vt_tiles = {}

            def u_vtr(tq, nh):
                """V^T -> Vp transposes, emitted a few units after their
                producer so they never wait at the head of the sync queue."""
                par = (tq // 2) % 2
                strip = (tq % 2) * 1024 + nh * 512

                def u():
                    vt = vt_tiles.pop((tq, nh))
                    kj0 = strip // 128
                    for j in range(4):
                        nc.sync.dma_start_transpose(
                            Vp[:, par, kj0 + j, :, :],
                            vt[:, j * 128:(j + 1) * 128],
                        )
                return u

            def qkv_units(tq):
                """K,V first (attention consumes all kj tiles in the first
                qi sweep of the next batch), Q strips last; each V strip's
                transposes trail its matmuls by two units."""
                return [u_qkv(1, tq, 0), u_qkv(1, tq, 1),
                        u_qkv(2, tq, 0), u_qkv(2, tq, 1),
                        u_qkv(0, tq, 0), u_vtr(tq, 0),
                        u_qkv(0, tq, 1), u_vtr(tq, 1)]

            dstage_t = {}  # (b, qi) -> [1,2,512] f32 denominators in SBUF

            def u_recip(b, qi):
                def u():
                    dst = dstage_t.pop((b, qi))
                    rf = work.tile([1, 2, 512], F32, tag="rf", bufs=2)
                    nc.vector.reciprocal_approx_fast(out=rf, in_=dst)
                    rb = work.tile([1, 2, 512], BF16, tag="rb", bufs=2)
                    nc.vector.tensor_copy(rb, rf)
                    nc.sync.dma_start(rden_d[b, qi], rb)
                return u

            def u_bcmul(b, qi):
                par = b % 2

                def u():
                    q0 = qi * 512
                    bc = work.tile([128, 512], BF16, tag="bc", bufs=2)
                    for h in range(HPC):
                        src = rden_d[b, qi, h, :]
                        bcast = bass.AP(tensor=src.tensor, offset=src.offset,
                                        ap=[[0, HD], [1, 512]])
                        nc.sync.dma_start(bc[h * HD:(h + 1) * HD, :], bcast)
                    nc.vector.tensor_mul(
                        attnT[:, par, q0:q0 + 512],
                        attnT[:, par, q0:q0 + 512],
                        bc,
                    )
                return u

            def u_a2a(b, half):
                par = b % 2

                def u():
                    base = attnT[:, par, half * HTOK:(half + 1) * HTOK]
                    nc.sync.dma_start(
                        ag_in[b, half].rearrange("j p c -> p j c"),
                        ap3(base, CTOK, NCORES, CTOK),
                    )
                    nc.gpsimd.collective_compute(
                        "AllToAll", mybir.AluOpType.bypass,
                        replica_groups=[list(range(NCORES))],
                        ins=[ag_in[b, half]], outs=[ag_out[b, half]],
                    )
                return u

            def proj_units(b, half):
                """Token-stationary projection of this core's 128 tokens of
                (b, half): out[tok, od] accumulated over the 8 rank blocks."""
                st = {}

                def u_dma():
                    agT = work.tile([128, 8, CTOK], BF16, tag="agT", bufs=2,
                                    name=f"agT{b}{half}")
                    nc.sync.dma_start(
                        agT, ag_out[b, half].rearrange("j p c -> p j c"))
                    st["agT"] = agT

                def mk_od(oh):
                    def u():
                        agT = st["agT"]
                        pp = ps_mm.tile([128, 512], F32, tag="mm",
                                        name=f"pp{b}{half}{oh}")
                        for r in range(8):
                            nc.tensor.matmul(
                                pp,
                                agT[:, r, :],
                                wproj_sb[:, r, oh * 512:(oh + 1) * 512],
                                start=(r == 0),
                                stop=(r == 7),
                            )
                        ob = work.tile([128, 512], F32, tag="ob", bufs=2,
                                       name=f"ob{b}{half}{oh}")
                        nc.vector.tensor_add(
                            ob, pp, biasb[:, oh * 512:(oh + 1) * 512])
                        nc.sync.dma_start(
                            out_tok[b, half, :, oh * 512:(oh + 1) * 512], ob)
                    return u

                return [u_dma, mk_od(0), mk_od(1)]

            # ================= main loop =================
            for b in range(B):
                par = b % 2
                t0 = 0  # attnT/QT/KT are parity-indexed, not batch-offset

                # -- scheduled inserts: step -> [units] --
                timeline = {}

                def put(step, *us):
                    timeline.setdefault(step, []).extend(us)

                if b == 0:
                    # minimal prologue: K, V (+transposes), Q for the first
                    # 512 tokens so attention step (0, 0) can start
                    u_qkv(1, 0, 0)()
                    u_qkv(2, 0, 0)()
                    u_qkv(0, 0, 0)()
                    u_vtr(0, 0)()
                    paced = [u_qkv(1, 0, 1), u_qkv(2, 0, 1), u_vtr(0, 1),
                             u_qkv(1, 1, 0), u_qkv(2, 1, 0), u_vtr(1, 0),
                             u_qkv(1, 1, 1), u_qkv(2, 1, 1), u_vtr(1, 1),
                             u_qkv(0, 0, 1), u_qkv(0, 1, 0), u_qkv(0, 1, 1)]
                    put(9, u_xdma(2))
                    put(21, u_xdma(3))
                    paced += qkv_units(2) + qkv_units(3)
                else:
                    paced = []
                    if b + 1 < B:
                        put(0, u_xdma(2 * b + 2))
                        put(2, u_xdma(2 * b + 3))
                        paced += qkv_units(2 * b + 2) + qkv_units(2 * b + 3)
                    # previous batch wind-down: last strip norm, half-1 A2A,
                    # then both halves' projections
                    put(2, u_recip(b - 1, 3))
                    put(4, u_bcmul(b - 1, 3))
                    put(6, u_a2a(b - 1, 1))
                    pu0 = proj_units(b - 1, 0)
                    put(10, pu0[0])
                    put(12, pu0[1])
                    put(14, pu0[2])
                    pu1 = proj_units(b - 1, 1)
                    put(26, pu1[0])
                    put(28, pu1[1])
                    put(30, pu1[2])

                # this batch's own norm + half-0 A2A
                put(17, u_recip(b, 0))
                put(19, u_bcmul(b, 0))
                put(33, u_recip(b, 1))
                put(35, u_bcmul(b, 1))
                put(38, u_a2a(b, 0))
                put(49, u_recip(b, 2))
                put(51, u_bcmul(b, 2))
                if b == B - 1:
                    pu = proj_units(b, 0)
                    put(56, pu[0])
                    put(58, pu[1])
                    put(60, pu[2])

                n_fill = len(paced)
                paced.reverse()  # pop() from the end = original order
                popped = 0

                steps = [(qi, kj) for qi in range(4) for kj in range(16)]
                pS_t = {}
                po_t = {}
                acc_t = {}

                def emit_S(qi, kj):
                    q0 = qi * 512
                    k0 = kj * 128
                    pS = ps_s.tile([128, 2, 512], F32, tag="s",
                                   name=f"pS{b}_{qi}_{kj}")
                    for h in range(HPC):
                        hs = h * HD
                        nc.tensor.matmul(
                            pS[:, h, :],
                            KT[hs:hs + HD, par, k0:k0 + 128],
                            QT[hs:hs + HD, par, q0:q0 + 512],
                            start=True,
                            stop=True,
                        )
                    pS_t[(qi, kj)] = pS

                emit_S(0, 0)
                for it, (qi, kj) in enumerate(steps):
                    q0 = qi * 512
                    if kj == 0:
                        po_t[qi] = ps_o.tile([128, 512], F32, tag="vo",
                                             name=f"po{b}_{qi}")
                        acc_t[qi] = [
                            work.tile([128, 2, 512], BF16, tag=f"acc{a}",
                                      name=f"acc{a}_{b}_{qi}")
                            for a in range(2)
                        ]
                    due = timeline.pop(it, [])
                    if b == 0:
                        target = ((it + 1) if it < 12
                                  else 12 + (it - 11) * (n_fill - 12) // 45)
                    else:
                        target = (it + 1) * n_fill // 56
                    # sandwich filler work around the sem-gated instructions
                    # (S waiting its PSUM slot, V waiting eS) so the in-order
                    # PE queue never idles at a blocked head
                    if due:
                        due[0]()
                        due = due[1:]
                    while paced and popped < min(target, n_fill):
                        paced.pop()()
                        popped += 1
                    if it + 1 < len(steps):
                        emit_S(*steps[it + 1])
                    pS = pS_t.pop((qi, kj))
                    eS = work.tile([128, 2, 512], BF16, tag="es", bufs=4)
                    nc.scalar.activation(eS, pS, EXP, scale=SCALE)
                    for u in due:
                        u()
                    po, acc = po_t[qi], acc_t[qi]
                    for h in range(HPC):
                        nc.tensor.matmul(
                            po[h * HD:(h + 1) * HD, :],
                            Vp[:, par, kj, h, :],
                            eS[:, h, :],
                            start=(kj == 0),
                            stop=(kj == 15),
                        )
                    a = kj // 8
                    if kj % 8 == 0:
                        nc.vector.tensor_copy(acc[a], eS)
                    else:
                        nc.vector.tensor_add(acc[a], acc[a], eS)
                    if kj == 15:
                        # stage numerators (unnormalized, both heads)
                        nc.vector.tensor_copy(
                            attnT[:, par, q0:q0 + 512], po)
                        # denominators: partition-reduce the accumulators
                        nc.vector.tensor_add(acc[0], acc[0], acc[1])
                        dst = work.tile([1, 2, 512], F32, tag="dst", bufs=4,
                                        name=f"dst{b}{qi}")
                        for h in range(HPC):
                            pden = ps_mm.tile([1, 512], F32, tag="mm",
                                              name=f"pden{b}{qi}{h}")
                            nc.tensor.matmul(pden, ones_sb[:, 0:1],
                                             acc[0][:, h, :],
                                             start=True, stop=True)
                            nc.vector.tensor_copy(dst[:, h, :], pden)
                        dstage_t[(b, qi)] = dst
                while paced:
                    paced.pop()()
                for s in sorted(timeline):
                    for u in timeline[s]:
                        u()

            # ---- tail: last batch, second half ----
            u_recip(B - 1, 3)()
            u_bcmul(B - 1, 3)()
            u_a2a(B - 1, 1)()
            for u in proj_units(B - 1, 1):
                u()

    nc.finalize()
    return nc


def kernel(x, w_qkv, w_proj, b_proj):
    global LAST_RESULTS
    bf16 = ml_dtypes.bfloat16

    x_t = np.ascontiguousarray(
        x.reshape(T, DIM).T.astype(bf16).reshape(8, 128, T))
    w_p = np.ascontiguousarray(w_proj.astype(bf16).reshape(8, 128, DIM))
    b_p = np.ascontiguousarray(b_proj.astype(np.float32))

    in_maps = []
    for c in range(NCORES):
        w_c = np.concatenate(
            [
                w_qkv[:, HC * c:HC * (c + 1)],
                w_qkv[:, DIM + HC * c:DIM + HC * (c + 1)],
                w_qkv[:, 2 * DIM + HC * c:2 * DIM + HC * (c + 1)],
            ],
            axis=1,
        ).astype(bf16).reshape(8, 128, 3 * HC)
        in_maps.append(
            {"x_t": x_t, "w_c": np.ascontiguousarray(w_c), "w_p": w_p,
             "b_p": b_p}
        )

    nc = _build()
    LAST_RESULTS = run_bass_kernel_spmd(
        nc, in_maps, core_ids=list(range(NCORES)),
        trace=bool(os.environ.get("KERNEL_TRACE")),
    )

    # core c's out_tok[b, hf] holds tokens [hf*1024 + c*128, +128) of batch b
    out = np.empty((B, N, DIM), dtype=np.float32)
    for c in range(NCORES):
        res = np.asarray(LAST_RESULTS.results[c]["out_tok"], dtype=np.float32)
        for b in range(B):
            for hf in range(2):
                o0 = hf * HTOK + c * CTOK
                out[b, o0:o0 + CTOK, :] = res[b, hf]
    return out
